# revision 1
# baseline (speedup 1.0000x reference)
"""Local (windowed) attention kernel for Trainium2, sequence-parallel over 8 NeuronCores.

Reference computation (fp32):
    qkv = x @ w_qkv ; q,k,v split, reshaped to (head, window, 128, 64)
    k,v get a 1-window zero-padded lookback -> (head, window, 256, 64)
    sim = q @ k.T * d^-0.5, causal-banded mask, softmax, out = attn @ v
    y = out @ w_out + b_out

Sharding: 128 windows of 128 tokens -> 16 windows per core, plus a 128-row
halo of x from the previous core (zeros for core 0, which exactly reproduces
the reference's zero-pad lookback including its effect on the softmax
denominator). No inter-core communication.

Device dataflow (per core, all bf16 matmuls accumulating in fp32):
  xT (host-pretransposed, [1024, 2176]) and w_qkv stream in; qkT = w_q/k.T @ xT
  keeps features on partitions, v = xT.T @ w_v keeps tokens on partitions with
  a ones-column appended per head (so attn@v also yields the softmax
  denominator for free). Scores are computed transposed (pT[j, i]) so that
  attn@v needs no on-device transposes and its output lands directly as the
  stationary operand of the output projection. Softmax skips max-subtraction
  (logits are ~N(0, 0.4); exp is safe in fp32).
"""

import sys

sys.path.insert(0, "/opt/trn_rl_repo")

import numpy as np
import ml_dtypes

import concourse.bass as bass
import concourse.mybir as mybir
import concourse.tile as tile
from concourse import bacc
from concourse.bass_utils import run_bass_kernel_spmd

BF16 = mybir.dt.bfloat16
F32 = mybir.dt.float32

N = 16384
DIM = 1024
HEADS = 8
DHEAD = 64
WSZ = 128
NCORES = 8
R = N // NCORES            # 2048 own rows per core
T = R + WSZ                # 2176 rows incl. halo
NW = R // WSZ              # 16 own windows
DK = DIM // 128            # 8 contraction chunks
P = 128
SCALE = DHEAD ** -0.5

_CACHE = {}


def _build():
    nc = bacc.Bacc()
    xT_d = nc.declare_dram_parameter("xT", [DIM, T], BF16, isOutput=False)
    wqkv_d = nc.declare_dram_parameter("wqkv", [DIM, 3 * HEADS * DHEAD], BF16, isOutput=False)
    wout_d = nc.declare_dram_parameter("wout", [HEADS * DHEAD, DIM], BF16, isOutput=False)
    maskT_d = nc.declare_dram_parameter("maskT", [P, P], BF16, isOutput=False)
    ones1_d = nc.declare_dram_parameter("ones1", [1, DHEAD], BF16, isOutput=False)
    out_d = nc.declare_dram_parameter("out", [R, DIM], F32, isOutput=True)

    # token blocks for the qkT projection (moving dim <= 512)
    tok_blocks = [(b, min(512, T - b)) for b in range(0, T, 512)]

    with tile.TileContext(nc) as tc:
        with (
            tc.tile_pool(name="pers", bufs=1) as pers,
            tc.tile_pool(name="work", bufs=3) as work,
            tc.tile_pool(name="outp", bufs=2) as outp,
            tc.tile_pool(name="ps512", bufs=2, space="PSUM") as ps512,
            tc.tile_pool(name="pspt", bufs=2, space="PSUM") as pspt,
            tc.tile_pool(name="pso", bufs=2, space="PSUM") as pso,
            tc.tile_pool(name="psb", bufs=2, space="PSUM") as psb,
        ):
            # ---- phase A: load inputs -------------------------------------
            xT_sb = [pers.tile([P, T], BF16, tag=f"xT{k}", name=f"xT{k}") for k in range(DK)]
            w_sb = [pers.tile([P, 3 * HEADS * DHEAD], BF16, tag=f"w{k}", name=f"w{k}") for k in range(DK)]
            wo_sb = [pers.tile([P, DIM], BF16, tag=f"wo{m}", name=f"wo{m}") for m in range(4)]
            maskT_sb = pers.tile([P, P], BF16, tag="maskT")
            ones1_sb = pers.tile([1, DHEAD], BF16, tag="ones1")
            for k in range(DK):
                nc.sync.dma_start(xT_sb[k][:], xT_d[k * P:(k + 1) * P, :])
                nc.sync.dma_start(w_sb[k][:], wqkv_d[k * P:(k + 1) * P, :])
            for m in range(4):
                nc.sync.dma_start(wo_sb[m][:], wout_d[m * P:(m + 1) * P, :])
            nc.sync.dma_start(maskT_sb[:], maskT_d[:])
            nc.sync.dma_start(ones1_sb[:], ones1_d[:])

            # ---- phase B: qkT[m] = w_qk[:, m-chunk].T @ xT  ([128, T]) ----
            qk_sb = [pers.tile([P, T], BF16, tag=f"qk{m}", name=f"qk{m}") for m in range(8)]
            for m in range(8):
                for (b0, bw) in tok_blocks:
                    pq = ps512.tile([P, 512], F32, tag="mm512", name="mm512")
                    for k in range(DK):
                        nc.tensor.matmul(
                            pq[:, :bw],
                            lhsT=w_sb[k][:, m * P:(m + 1) * P],
                            rhs=xT_sb[k][:, b0:b0 + bw],
                            start=(k == 0), stop=(k == DK - 1),
                        )
                    nc.vector.tensor_copy(qk_sb[m][:, b0:b0 + bw], pq[:, :bw])

            # ---- phase C: v[t] = xT[:, t-tile].T @ w_v  (+ ones column) ---
            # v_sb[t] is [128 tok, 8 heads, 65]; [:, h, 0:64] = v, [:, h, 64] = 1
            v_sb = [pers.tile([P, HEADS, DHEAD + 1], BF16, tag=f"v{t}", name=f"v{t}") for t in range(NW + 1)]
            for t in range(NW + 1):
                nc.vector.memset(v_sb[t][:, :, DHEAD:DHEAD + 1], 1.0)
                pv = ps512.tile([P, 512], F32, tag="mm512", name="mm512")
                for k in range(DK):
                    nc.tensor.matmul(
                        pv[:],
                        lhsT=xT_sb[k][:, t * P:(t + 1) * P],
                        rhs=w_sb[k][:, 1024:1536],
                        start=(k == 0), stop=(k == DK - 1),
                    )
                nc.vector.tensor_copy(
                    v_sb[t][:, :, 0:DHEAD],
                    pv.rearrange("p (h d) -> p h d", h=HEADS),
                )

            # ---- phase D: attention per (window, head) --------------------
            # attn_sb[m] rows 0:64 = head 2m, 64:128 = head 2m+1 (out.T layout)
            attn_sb = [pers.tile([P, R], BF16, tag=f"at{m}", name=f"at{m}") for m in range(4)]
            for w in range(NW):
                for h in range(HEADS):
                    mq, off = h // 2, (h % 2) * 64
                    mk = 4 + h // 2
                    i0 = (w + 1) * P
                    ppt = pspt.tile([P, 2, P], F32, tag="pT", name="pT")
                    # scores transposed: pT[j, i] for j in prev/cur window
                    for jc in range(2):
                        j0 = (w + jc) * P
                        nc.tensor.matmul(
                            ppt[:, jc, :],
                            lhsT=qk_sb[mk][off:off + 64, j0:j0 + P],
                            rhs=qk_sb[mq][off:off + 64, i0:i0 + P],
                            start=True, stop=True,
                        )
                    pt_sb = work.tile([P, 2, P], BF16, tag="pt_sb", name="pt_sb")
                    nc.scalar.activation(pt_sb[:], ppt[:],
                                         mybir.ActivationFunctionType.Exp, scale=SCALE)
                    # causal mask inside the current window (prev window is
                    # fully visible: j <= i + 128 always holds there)
                    nc.vector.tensor_mul(pt_sb[:, 1, :], pt_sb[:, 1, :], maskT_sb[:])
                    # attn @ v (+ denominator in row 64, from the ones column)
                    po = pso.tile([DHEAD + 1, P], F32, tag="o", name="po")
                    for jc in range(2):
                        nc.tensor.matmul(
                            po[:],
                            lhsT=v_sb[w + jc][:, h, :],
                            rhs=pt_sb[:, jc, :],
                            start=(jc == 0), stop=(jc == 1),
                        )
                    r_sb = work.tile([1, P], BF16, tag="r_sb", name="r_sb")
                    with nc.allow_low_precision(reason="softmax denom recip in bf16"):
                        nc.vector.reciprocal(r_sb[:], po[DHEAD:DHEAD + 1, :])
                    # broadcast recip across 64 partitions via K=1 outer product
                    pb = psb.tile([DHEAD, P], F32, tag="b", name="pb")
                    nc.tensor.matmul(pb[:], lhsT=ones1_sb[:], rhs=r_sb[:],
                                     start=True, stop=True)
                    b_sb = work.tile([DHEAD, P], F32, tag="b_sb", name="b_sb")
                    nc.scalar.copy(b_sb[:], pb[:])
                    nc.vector.tensor_mul(
                        attn_sb[mq][off:off + 64, w * P:(w + 1) * P],
                        po[0:DHEAD, :], b_sb[:],
                    )

            # ---- phase E: out = attn.T @ w_out ----------------------------
            for t in range(NW):
                o_sb = outp.tile([P, DIM], F32, tag="o_sb", name="o_sb")
                for nf in range(2):
                    pf = ps512.tile([P, 512], F32, tag="mm512", name="mm512")
                    for m in range(4):
                        nc.tensor.matmul(
                            pf[:],
                            lhsT=attn_sb[m][:, t * P:(t + 1) * P],
                            rhs=wo_sb[m][:, nf * 512:(nf + 1) * 512],
                            start=(m == 0), stop=(m == 3),
                        )
                    nc.vector.tensor_copy(o_sb[:, nf * 512:(nf + 1) * 512], pf[:])
                nc.sync.dma_start(out_d[t * P:(t + 1) * P, :], o_sb[:])

    nc.compile()
    return nc


def _get_nc():
    if "nc" not in _CACHE:
        _CACHE["nc"] = _build()
    return _CACHE["nc"]


def kernel(x, w_qkv, w_out, b_out):
    x = np.asarray(x, dtype=np.float32)
    w_qkv_b = np.asarray(w_qkv, dtype=np.float32).astype(ml_dtypes.bfloat16)
    w_out_b = np.asarray(w_out, dtype=np.float32).astype(ml_dtypes.bfloat16)
    b_out = np.asarray(b_out, dtype=np.float32)

    # maskT[j, i] = 1 where j <= i  (transposed causal mask for current window)
    maskT = np.triu(np.ones((P, P), dtype=np.float32)).astype(ml_dtypes.bfloat16)
    ones1 = np.ones((1, DHEAD), dtype=ml_dtypes.bfloat16)

    x_pad = np.concatenate([np.zeros((WSZ, DIM), np.float32), x], axis=0)
    in_maps = []
    for c in range(NCORES):
        x_sh = x_pad[c * R:c * R + T]                       # (2176, 1024)
        xT = np.ascontiguousarray(x_sh.T).astype(ml_dtypes.bfloat16)
        in_maps.append({
            "xT": xT,
            "wqkv": w_qkv_b,
            "wout": w_out_b,
            "maskT": maskT,
            "ones1": ones1,
        })

    nc = _get_nc()
    res = run_bass_kernel_spmd(nc, in_maps, core_ids=list(range(NCORES)))
    out = np.concatenate([res.results[c]["out"] for c in range(NCORES)], axis=0)
    return out + b_out[None, :]



# revision 16
# speedup vs baseline: 1.9088x; 1.9088x over previous
"""Local (windowed) attention kernel for Trainium2, sequence-parallel over 8 NeuronCores.

Reference computation (fp32):
    qkv = x @ w_qkv ; q,k,v split, reshaped to (head, window, 128, 64)
    k,v get a 1-window zero-padded lookback -> (head, window, 256, 64)
    sim = q @ k.T * d^-0.5, causal-banded mask, softmax, out = attn @ v
    y = out @ w_out + b_out

Sharding: 128 windows of 128 tokens -> 16 windows per core, plus a 128-row
halo of x from the previous core (zeros for core 0, which exactly reproduces
the reference's zero-pad lookback including its effect on the softmax
denominator). No inter-core communication.

Device dataflow (per core, all bf16 matmuls accumulating in fp32):
  Phase B: qkT[m] = w_qk[:,m].T @ xT keeps q/k features on partitions.
  Phase C: v = xT.T @ w_v keeps tokens on partitions, with a ones-column
    appended per head so attn@v also emits the softmax denominator.
  Phase D: scores are computed transposed (pT[j,i]); exp on the Act engine
    (no max-subtraction: logits ~N(0,0.4)); causal mask of the current
    window by one batched DVE multiply; attn@v with pT as the *stationary*
    operand so the output lands tokens-on-partitions [i, d | denom] and the
    whole softmax normalization is a single GPSIMD normalize_recip per
    head; PE-transpose back to [hd, tokens] for the projection.
    NOTE: matmuls with different PE tile_position row offsets (heads with
    q/k features on partitions 0:64 vs 64:128) must not share a PSUM bank
    — each score bank groups two same-offset heads.
  Phase E: y = attnT.T @ w_out, written out in bf16.
Emission is software-pipelined (S/A/T/E stages over windows) and interleaved
with the projection rounds so PE stays busy and Act/DVE/Pool run in parallel.
"""

import sys

sys.path.insert(0, "/opt/trn_rl_repo")

import numpy as np
import ml_dtypes

import concourse.bass as bass
import concourse.mybir as mybir
import concourse.tile as tile
from concourse import bacc
from concourse.bass_utils import run_bass_kernel_spmd

BF16 = mybir.dt.bfloat16
F32 = mybir.dt.float32

N = 16384
DIM = 1024
HEADS = 8
DHEAD = 64
WSZ = 128
NCORES = 8
R = N // NCORES            # 2048 own rows per core
T = R + WSZ                # 2176 rows incl. halo
NW = R // WSZ              # 16 own windows
DK = DIM // 128            # 8 contraction chunks
P = 128
SCALE = DHEAD ** -0.5

# score-tile head groups: two heads per PSUM bank, same q/k partition offset
SGROUPS = [(0, 2), (1, 3), (4, 6), (5, 7)]
GRP_OF = {h: g for g, hs in enumerate(SGROUPS) for h in hs}
IDX_OF = {h: e for hs in SGROUPS for e, h in enumerate(hs)}

_CACHE = {}


def _build():
    nc = bacc.Bacc()
    xT_d = nc.declare_dram_parameter("xT", [DIM, T], BF16, isOutput=False)
    wqkv_d = nc.declare_dram_parameter("wqkv", [DIM, 3 * HEADS * DHEAD], BF16, isOutput=False)
    wout_d = nc.declare_dram_parameter("wout", [HEADS * DHEAD, DIM], BF16, isOutput=False)
    mask2_d = nc.declare_dram_parameter("mask2", [P, 2, P], BF16, isOutput=False)
    ident_d = nc.declare_dram_parameter("ident", [P, P], BF16, isOutput=False)
    out_d = nc.declare_dram_parameter("out", [R, DIM], BF16, isOutput=True)

    # projection column rounds: q only needs own tokens, k needs the halo too
    qblocks = [(WSZ + r * 512, 512) for r in range(4)]
    kblocks = [(0, 512), (512, 512), (1024, 512), (1536, 512), (2048, 128)]

    with tile.TileContext(nc) as tc:
        with (
            tc.tile_pool(name="pers", bufs=1) as pers,
            tc.tile_pool(name="ptp", bufs=8) as ptp,
            tc.tile_pool(name="osbp", bufs=4) as osbp,
            tc.tile_pool(name="atokp", bufs=3) as atokp,
            tc.tile_pool(name="eop", bufs=3) as eop,
            tc.tile_pool(name="ps512", bufs=2, space="PSUM") as ps512,
            tc.tile_pool(name="psS", bufs=3, space="PSUM") as psS,
            tc.tile_pool(name="psP", bufs=2, space="PSUM") as psP,
            tc.tile_pool(name="psT", bufs=1, space="PSUM") as psT,
        ):
            # ---- persistent SBUF tiles ------------------------------------
            xT_sb = [pers.tile([P, T], BF16, tag=f"xT{k}", name=f"xT{k}") for k in range(DK)]
            w_sb = [pers.tile([P, 3 * HEADS * DHEAD], BF16, tag=f"w{k}", name=f"w{k}") for k in range(DK)]
            wo_sb = [pers.tile([P, DIM], BF16, tag=f"wo{m}", name=f"wo{m}") for m in range(4)]
            qk_sb = [pers.tile([P, T], BF16, tag=f"qk{m}", name=f"qk{m}") for m in range(8)]
            v_sb = [pers.tile([P, HEADS, DHEAD + 1], BF16, tag=f"v{t}", name=f"v{t}") for t in range(NW + 1)]
            attnT_sb = pers.tile([P, 4, R], BF16, tag="attnT")
            mask2_sb = pers.tile([P, 2, P], BF16, tag="mask2")
            ident_sb = pers.tile([P, P], BF16, tag="ident")

            # ---- input DMAs -----------------------------------------------
            for k in range(DK):
                nc.sync.dma_start(xT_sb[k][:], xT_d[k * P:(k + 1) * P, :])
                nc.sync.dma_start(w_sb[k][:], wqkv_d[k * P:(k + 1) * P, :])
            for m in range(4):
                nc.sync.dma_start(wo_sb[m][:], wout_d[m * P:(m + 1) * P, :])
            nc.sync.dma_start(mask2_sb[:], mask2_d[:])
            nc.sync.dma_start(ident_sb[:], ident_d[:])
            for t in range(NW + 1):
                nc.vector.memset(v_sb[t][:, :, DHEAD:DHEAD + 1], 1.0)

            # ---- stage emitters -------------------------------------------
            def emit_Bq(m, blk):
                b0, bw = blk
                pq = ps512.tile([P, 512], F32, tag="mm512", name="pq")
                for k in range(DK):
                    nc.tensor.matmul(
                        pq[:, :bw],
                        lhsT=w_sb[k][:, m * P:(m + 1) * P],
                        rhs=xT_sb[k][:, b0:b0 + bw],
                        start=(k == 0), stop=(k == DK - 1),
                    )
                nc.vector.tensor_copy(qk_sb[m][:, b0:b0 + bw], pq[:, :bw])

            def emit_Bk(m, blk):
                b0, bw = blk
                pk = ps512.tile([P, 512], F32, tag="mm512", name="pk")
                for k in range(DK):
                    nc.tensor.matmul(
                        pk[:, :bw],
                        lhsT=w_sb[k][:, m * P:(m + 1) * P],
                        rhs=xT_sb[k][:, b0:b0 + bw],
                        start=(k == 0), stop=(k == DK - 1),
                    )
                nc.scalar.copy(qk_sb[m][:, b0:b0 + bw], pk[:, :bw])

            def emit_C(t):
                pv = ps512.tile([P, 512], F32, tag="mm512", name="pv")
                for k in range(DK):
                    nc.tensor.matmul(
                        pv[:],
                        lhsT=xT_sb[k][:, t * P:(t + 1) * P],
                        rhs=w_sb[k][:, 1024:1536],
                        start=(k == 0), stop=(k == DK - 1),
                    )
                nc.scalar.copy(
                    v_sb[t][:, :, 0:DHEAD],
                    pv.rearrange("p (h d) -> p h d", h=HEADS),
                )

            def emit_round(r):
                if r < 4:
                    for m in range(4):
                        emit_Bq(m, qblocks[r])
                for m in range(4, 8):
                    emit_Bk(m, kblocks[r])
                if r < 4:
                    for t in range(4 * r, 4 * r + 4):
                        emit_C(t)
                else:
                    emit_C(NW)

            # S: transposed scores + exp + mask for head group g (see SGROUPS)
            def emit_S(w, g):
                i0 = (w + 1) * P
                off = (g % 2) * DHEAD
                ps = psS.tile([P, 2, 2, P], F32, tag="sc", name="ps")
                for e, h in enumerate(SGROUPS[g]):
                    a = h // 2
                    for jc in range(2):
                        j0 = (w + jc) * P
                        nc.tensor.matmul(
                            ps[:, e, jc, :],
                            lhsT=qk_sb[4 + a][off:off + DHEAD, j0:j0 + P],
                            rhs=qk_sb[a][off:off + DHEAD, i0:i0 + P],
                            start=True, stop=True,
                        )
                pt = ptp.tile([P, 2, 2, P], BF16, tag="pt", name="pt")
                nc.scalar.activation(pt.rearrange("p a b i -> p (a b i)"),
                                     ps.rearrange("p a b i -> p (a b i)"),
                                     mybir.ActivationFunctionType.Exp, scale=SCALE)
                # causal mask inside the current window (prev window fully visible)
                nc.vector.tensor_mul(pt[:, :, 1, :], pt[:, :, 1, :], mask2_sb[:])
                return pt

            # A: attn@v for heads 4b..4b+3, tokens-on-partitions + normalize
            def emit_A(w, b, pts, atok):
                # po head slots padded to 512B so matmul outputs stay
                # bank-row aligned in PSUM
                po = psP.tile([P, 4, P], F32, tag="po", name="po")
                for hh in range(4):
                    h = 4 * b + hh
                    pt = pts[GRP_OF[h]]
                    e = IDX_OF[h]
                    for jc in range(2):
                        nc.tensor.matmul(
                            po[:, hh, 0:DHEAD + 1],
                            lhsT=pt[:, e, jc, :],
                            rhs=v_sb[w + jc][:, h, :],
                            start=(jc == 0), stop=(jc == 1),
                        )
                osb = osbp.tile([P, 4, DHEAD + 1], F32, tag="osb", name="osb")
                nc.scalar.copy(osb[:], po[:, :, 0:DHEAD + 1])
                for hh in range(4):
                    nc.gpsimd.normalize_recip(
                        atok[:, 4 * b + hh, :],
                        osb[:, hh, 0:DHEAD],
                        osb[:, hh, DHEAD:DHEAD + 1],
                    )

            # T: PE-transpose normalized attn back to [hd, tokens].
            # A regular matmul lhsT.T @ I is an exact transpose for bf16
            # inputs (f32 accumulate is lossless), avoiding is_transpose mode.
            def emit_T(w, atok):
                pT = psT.tile([P, 4, P], F32, tag="tr", name="pT")
                for m in range(4):
                    nc.tensor.matmul(
                        pT[:, m, :],
                        lhsT=atok[:, 2 * m:2 * m + 2, :],
                        rhs=ident_sb[:],
                        start=True, stop=True,
                    )
                nc.vector.tensor_copy(attnT_sb[:, :, w * P:(w + 1) * P], pT[:])

            # E: output projection for one window + store
            def emit_E(w):
                eo = eop.tile([P, DIM], BF16, tag="eo", name="eo")
                for nf in range(2):
                    pf = ps512.tile([P, 512], F32, tag="mm512", name="pf")
                    for m in range(4):
                        nc.tensor.matmul(
                            pf[:],
                            lhsT=attnT_sb[:, m, w * P:(w + 1) * P],
                            rhs=wo_sb[m][:, nf * 512:(nf + 1) * 512],
                            start=(m == 0), stop=(m == 3),
                        )
                    if nf == 0:
                        nc.vector.tensor_copy(eo[:, 0:512], pf[:])
                    else:
                        nc.scalar.copy(eo[:, 512:1024], pf[:])
                nc.sync.dma_start(out_d[w * P:(w + 1) * P, :], eo[:])

            # ---- software-pipelined schedule ------------------------------
            # step i: S(i) | A(i-1) | T(i-2) | E(i-3); projection rounds are
            # injected so every window's inputs are emitted a round ahead.
            round_before_step = {0: [0, 1], 5: [2], 9: [3], 13: [4]}
            pts_of = {}
            atok_of = {}
            for i in range(NW + 3):
                for r in round_before_step.get(i, []):
                    emit_round(r)
                wS, wA, wT, wE = i, i - 1, i - 2, i - 3
                if wA in pts_of:
                    atok_of[wA] = atokp.tile([P, HEADS, DHEAD], BF16, tag="atok", name="atok")
                if wS <= NW - 1:
                    pts = [emit_S(wS, 0), emit_S(wS, 1)]
                    if wA in pts_of:
                        emit_A(wA, 0, pts_of[wA], atok_of[wA])
                    pts += [emit_S(wS, 2), emit_S(wS, 3)]
                    if wA in pts_of:
                        emit_A(wA, 1, pts_of[wA], atok_of[wA])
                    pts_of[wS] = pts
                elif wA in pts_of:
                    emit_A(wA, 0, pts_of[wA], atok_of[wA])
                    emit_A(wA, 1, pts_of[wA], atok_of[wA])
                if wA in pts_of:
                    del pts_of[wA]
                if wT in atok_of:
                    emit_T(wT, atok_of[wT])
                    del atok_of[wT]
                if 0 <= wE:
                    emit_E(wE)

    nc.compile()
    return nc


def _get_nc():
    if "nc" not in _CACHE:
        _CACHE["nc"] = _build()
    return _CACHE["nc"]


def make_in_maps(x, w_qkv, w_out):
    x = np.asarray(x, dtype=np.float32)
    w_qkv_b = np.asarray(w_qkv, dtype=np.float32).astype(ml_dtypes.bfloat16)
    w_out_b = np.asarray(w_out, dtype=np.float32).astype(ml_dtypes.bfloat16)

    # mask2[j, c, i] = 1 where j <= i, replicated for both heads of a group
    maskT = np.triu(np.ones((P, P), dtype=np.float32))
    mask2 = np.broadcast_to(maskT[:, None, :], (P, 2, P)).astype(ml_dtypes.bfloat16)
    mask2 = np.ascontiguousarray(mask2)
    ident = np.eye(P, dtype=np.float32).astype(ml_dtypes.bfloat16)

    x_pad = np.concatenate([np.zeros((WSZ, DIM), np.float32), x], axis=0)
    in_maps = []
    for c in range(NCORES):
        x_sh = x_pad[c * R:c * R + T]                       # (2176, 1024)
        xT = np.ascontiguousarray(x_sh.T).astype(ml_dtypes.bfloat16)
        in_maps.append({
            "xT": xT,
            "wqkv": w_qkv_b,
            "wout": w_out_b,
            "mask2": mask2,
            "ident": ident,
        })
    return in_maps


def kernel(x, w_qkv, w_out, b_out):
    b_out = np.asarray(b_out, dtype=np.float32)
    in_maps = make_in_maps(x, w_qkv, w_out)
    nc = _get_nc()
    res = run_bass_kernel_spmd(nc, in_maps, core_ids=list(range(NCORES)))
    out = np.concatenate(
        [res.results[c]["out"].astype(np.float32) for c in range(NCORES)], axis=0
    )
    return out + b_out[None, :]


# revision 22
# speedup vs baseline: 1.9153x; 1.0034x over previous
"""Local (windowed) attention kernel for Trainium2, sequence-parallel over 8 NeuronCores.

Reference computation (fp32):
    qkv = x @ w_qkv ; q,k,v split, reshaped to (head, window, 128, 64)
    k,v get a 1-window zero-padded lookback -> (head, window, 256, 64)
    sim = q @ k.T * d^-0.5, causal-banded mask, softmax, out = attn @ v
    y = out @ w_out + b_out

Sharding: 128 windows of 128 tokens -> 16 windows per core, plus a 128-row
halo of x from the previous core (zeros for core 0, which exactly reproduces
the reference's zero-pad lookback including its effect on the softmax
denominator). No inter-core communication.

Device dataflow (per core, all bf16 matmuls accumulating in fp32):
  Phase B: qkT[m] = w_qk[:,m].T @ xT keeps q/k features on partitions.
  Phase C: v = xT.T @ w_v keeps tokens on partitions, with a ones-column
    appended per head so attn@v also emits the softmax denominator.
  Phase D: scores are computed transposed (pT[j,i]); exp on the Act engine
    (no max-subtraction: logits ~N(0,0.4)); causal mask of the current
    window by one batched DVE multiply; attn@v with pT as the *stationary*
    operand so the output lands tokens-on-partitions [i, d | denom] and the
    whole softmax normalization is a single GPSIMD normalize_recip per
    head; PE-transpose back to [hd, tokens] for the projection.
    NOTE: matmuls with different PE tile_position row offsets (heads with
    q/k features on partitions 0:64 vs 64:128) must not share a PSUM bank
    — each score bank groups two same-offset heads.
  Phase E: y = attnT.T @ w_out, written out in bf16.
Emission is software-pipelined (S/A/T/E stages over windows) and interleaved
with the projection rounds so PE stays busy and Act/DVE/Pool run in parallel.
"""

import sys

sys.path.insert(0, "/opt/trn_rl_repo")

import numpy as np
import ml_dtypes

import concourse.bass as bass
import concourse.mybir as mybir
import concourse.tile as tile
from concourse import bacc
from concourse.bass_utils import run_bass_kernel_spmd

BF16 = mybir.dt.bfloat16
F32 = mybir.dt.float32

N = 16384
DIM = 1024
HEADS = 8
DHEAD = 64
WSZ = 128
NCORES = 8
R = N // NCORES            # 2048 own rows per core
T = R + WSZ                # 2176 rows incl. halo
NW = R // WSZ              # 16 own windows
DK = DIM // 128            # 8 contraction chunks
P = 128
SCALE = DHEAD ** -0.5

# score-tile head groups: two heads per PSUM bank, same q/k partition offset
SGROUPS = [(0, 2), (1, 3), (4, 6), (5, 7)]
GRP_OF = {h: g for g, hs in enumerate(SGROUPS) for h in hs}
IDX_OF = {h: e for hs in SGROUPS for e, h in enumerate(hs)}

_CACHE = {}


def _build():
    nc = bacc.Bacc()
    xT_d = nc.declare_dram_parameter("xT", [DIM, T], BF16, isOutput=False)
    wqkv_d = nc.declare_dram_parameter("wqkv", [DIM, 3 * HEADS * DHEAD], BF16, isOutput=False)
    wout_d = nc.declare_dram_parameter("wout", [HEADS * DHEAD, DIM], BF16, isOutput=False)
    mask2_d = nc.declare_dram_parameter("mask2", [P, 2, P], BF16, isOutput=False)
    ident_d = nc.declare_dram_parameter("ident", [P, P], BF16, isOutput=False)
    out_d = nc.declare_dram_parameter("out", [R, DIM], BF16, isOutput=True)

    # projection column rounds: q only needs own tokens, k needs the halo too
    qblocks = [(WSZ + r * 512, 512) for r in range(4)]
    kblocks = [(0, 512), (512, 512), (1024, 512), (1536, 512), (2048, 128)]

    with tile.TileContext(nc) as tc:
        with (
            tc.tile_pool(name="pers", bufs=1) as pers,
            tc.tile_pool(name="ptp", bufs=8) as ptp,
            tc.tile_pool(name="osbp", bufs=4) as osbp,
            tc.tile_pool(name="atokp", bufs=3) as atokp,
            tc.tile_pool(name="eop", bufs=3) as eop,
            tc.tile_pool(name="ps512", bufs=2, space="PSUM") as ps512,
            tc.tile_pool(name="psS", bufs=3, space="PSUM") as psS,
            tc.tile_pool(name="psP", bufs=2, space="PSUM") as psP,
            tc.tile_pool(name="psT", bufs=1, space="PSUM") as psT,
        ):
            # ---- persistent SBUF tiles ------------------------------------
            xT_all = pers.tile([P, DK, T], BF16, tag="xT")
            w_all = pers.tile([P, DK, 3 * HEADS * DHEAD], BF16, tag="w")
            wo_all = pers.tile([P, 4, DIM], BF16, tag="wo")
            xT_sb = [xT_all[:, k, :] for k in range(DK)]
            w_sb = [w_all[:, k, :] for k in range(DK)]
            wo_sb = [wo_all[:, m, :] for m in range(4)]
            qk_sb = [pers.tile([P, T], BF16, tag=f"qk{m}", name=f"qk{m}") for m in range(8)]
            v_sb = [pers.tile([P, HEADS, DHEAD + 1], BF16, tag=f"v{t}", name=f"v{t}") for t in range(NW + 1)]
            attnT_sb = pers.tile([P, 4, R], BF16, tag="attnT")
            mask2_sb = pers.tile([P, 2, P], BF16, tag="mask2")
            ident_sb = pers.tile([P, P], BF16, tag="ident")

            # ---- input DMAs: few fat transfers, issued from 3 queues ------
            xT_dv = xT_d.rearrange("(k p) t -> p k t", p=P)
            w_dv = wqkv_d.rearrange("(k p) c -> p k c", p=P)
            wo_dv = wout_d.rearrange("(m p) c -> p m c", p=P)
            for kk in range(4):
                nc.sync.dma_start(xT_all[:, 2 * kk:2 * kk + 2, :], xT_dv[:, 2 * kk:2 * kk + 2, :])
                nc.scalar.dma_start(w_all[:, 2 * kk:2 * kk + 2, :], w_dv[:, 2 * kk:2 * kk + 2, :])
            nc.gpsimd.dma_start(wo_all[:], wo_dv[:])
            nc.gpsimd.dma_start(mask2_sb[:], mask2_d[:])
            nc.gpsimd.dma_start(ident_sb[:], ident_d[:])
            for t in range(NW + 1):
                nc.vector.memset(v_sb[t][:, :, DHEAD:DHEAD + 1], 1.0)

            # ---- stage emitters -------------------------------------------
            def emit_Bq(m, blk):
                b0, bw = blk
                pq = ps512.tile([P, 512], F32, tag="mm512", name="pq")
                for k in range(DK):
                    nc.tensor.matmul(
                        pq[:, :bw],
                        lhsT=w_sb[k][:, m * P:(m + 1) * P],
                        rhs=xT_sb[k][:, b0:b0 + bw],
                        start=(k == 0), stop=(k == DK - 1),
                    )
                nc.vector.tensor_copy(qk_sb[m][:, b0:b0 + bw], pq[:, :bw])

            def emit_Bk(m, blk):
                b0, bw = blk
                pk = ps512.tile([P, 512], F32, tag="mm512", name="pk")
                for k in range(DK):
                    nc.tensor.matmul(
                        pk[:, :bw],
                        lhsT=w_sb[k][:, m * P:(m + 1) * P],
                        rhs=xT_sb[k][:, b0:b0 + bw],
                        start=(k == 0), stop=(k == DK - 1),
                    )
                nc.scalar.copy(qk_sb[m][:, b0:b0 + bw], pk[:, :bw])

            def emit_C(t):
                pv = ps512.tile([P, 512], F32, tag="mm512", name="pv")
                for k in range(DK):
                    nc.tensor.matmul(
                        pv[:],
                        lhsT=xT_sb[k][:, t * P:(t + 1) * P],
                        rhs=w_sb[k][:, 1024:1536],
                        start=(k == 0), stop=(k == DK - 1),
                    )
                nc.scalar.copy(
                    v_sb[t][:, :, 0:DHEAD],
                    pv.rearrange("p (h d) -> p h d", h=HEADS),
                )

            def emit_half_round(r, half):
                ms = (0, 1) if half == 0 else (2, 3)
                if r < 4:
                    for m in ms:
                        emit_Bq(m, qblocks[r])
                for m in ms:
                    emit_Bk(4 + m, kblocks[r])
                if r < 4:
                    for t in (4 * r + 2 * half, 4 * r + 2 * half + 1):
                        emit_C(t)
                elif half == 1:
                    emit_C(NW)

            # S: transposed scores + exp + mask for head group g (see SGROUPS)
            def emit_S(w, g):
                i0 = (w + 1) * P
                off = (g % 2) * DHEAD
                ps = psS.tile([P, 2, 2, P], F32, tag="sc", name="ps")
                for e, h in enumerate(SGROUPS[g]):
                    a = h // 2
                    for jc in range(2):
                        j0 = (w + jc) * P
                        nc.tensor.matmul(
                            ps[:, e, jc, :],
                            lhsT=qk_sb[4 + a][off:off + DHEAD, j0:j0 + P],
                            rhs=qk_sb[a][off:off + DHEAD, i0:i0 + P],
                            start=True, stop=True,
                        )
                pt = ptp.tile([P, 2, 2, P], BF16, tag="pt", name="pt")
                nc.scalar.activation(pt.rearrange("p a b i -> p (a b i)"),
                                     ps.rearrange("p a b i -> p (a b i)"),
                                     mybir.ActivationFunctionType.Exp, scale=SCALE)
                # causal mask inside the current window (prev window fully visible)
                nc.vector.tensor_mul(pt[:, :, 1, :], pt[:, :, 1, :], mask2_sb[:])
                return pt

            # A: attn@v for heads 4b..4b+3, tokens-on-partitions + normalize
            def emit_A(w, b, pts, atok):
                # po head slots padded to 512B so matmul outputs stay
                # bank-row aligned in PSUM
                po = psP.tile([P, 4, P], F32, tag="po", name="po")
                for hh in range(4):
                    h = 4 * b + hh
                    pt = pts[GRP_OF[h]]
                    e = IDX_OF[h]
                    for jc in range(2):
                        nc.tensor.matmul(
                            po[:, hh, 0:DHEAD + 1],
                            lhsT=pt[:, e, jc, :],
                            rhs=v_sb[w + jc][:, h, :],
                            start=(jc == 0), stop=(jc == 1),
                        )
                osb = osbp.tile([P, 4, DHEAD + 1], F32, tag="osb", name="osb")
                nc.vector.tensor_copy(osb[:], po[:, :, 0:DHEAD + 1])
                for hh in range(4):
                    nc.gpsimd.normalize_recip(
                        atok[:, 4 * b + hh, :],
                        osb[:, hh, 0:DHEAD],
                        osb[:, hh, DHEAD:DHEAD + 1],
                    )

            # T: PE-transpose normalized attn back to [hd, tokens].
            # A regular matmul lhsT.T @ I is an exact transpose for bf16
            # inputs (f32 accumulate is lossless), avoiding is_transpose mode.
            def emit_T(w, atok):
                pT = psT.tile([P, 4, P], F32, tag="tr", name="pT")
                for m in range(4):
                    nc.tensor.matmul(
                        pT[:, m, :],
                        lhsT=atok[:, 2 * m:2 * m + 2, :],
                        rhs=ident_sb[:],
                        start=True, stop=True,
                    )
                nc.vector.tensor_copy(attnT_sb[:, :, w * P:(w + 1) * P], pT[:])

            # E: output projection for one window + store
            def emit_E(w):
                eo = eop.tile([P, DIM], BF16, tag="eo", name="eo")
                for nf in range(2):
                    pf = ps512.tile([P, 512], F32, tag="mm512", name="pf")
                    for m in range(4):
                        nc.tensor.matmul(
                            pf[:],
                            lhsT=attnT_sb[:, m, w * P:(w + 1) * P],
                            rhs=wo_sb[m][:, nf * 512:(nf + 1) * 512],
                            start=(m == 0), stop=(m == 3),
                        )
                    nc.vector.tensor_copy(eo[:, nf * 512:(nf + 1) * 512], pf[:])
                nc.sync.dma_start(out_d[w * P:(w + 1) * P, :], eo[:])

            # ---- software-pipelined schedule ------------------------------
            # step i: S(i) | A(i-1) | T(i-2) | E(i-3); projection rounds are
            # injected so every window's inputs are emitted a round ahead.
            halves_before_step = {
                0: [(0, 0), (0, 1), (1, 0), (1, 1)],
                4: [(2, 0)], 5: [(2, 1)],
                8: [(3, 0)], 9: [(3, 1)],
                12: [(4, 0)], 13: [(4, 1)],
            }
            pts_of = {}
            atok_of = {}
            for i in range(NW + 3):
                for (r, half) in halves_before_step.get(i, []):
                    emit_half_round(r, half)
                wS, wA, wT, wE = i, i - 1, i - 2, i - 3
                if wA in pts_of:
                    atok_of[wA] = atokp.tile([P, HEADS, DHEAD], BF16, tag="atok", name="atok")
                if wS <= NW - 1:
                    pts = [emit_S(wS, 0), emit_S(wS, 1)]
                    if wA in pts_of:
                        emit_A(wA, 0, pts_of[wA], atok_of[wA])
                    pts += [emit_S(wS, 2), emit_S(wS, 3)]
                    if wA in pts_of:
                        emit_A(wA, 1, pts_of[wA], atok_of[wA])
                    pts_of[wS] = pts
                elif wA in pts_of:
                    emit_A(wA, 0, pts_of[wA], atok_of[wA])
                    emit_A(wA, 1, pts_of[wA], atok_of[wA])
                if wA in pts_of:
                    del pts_of[wA]
                if wT in atok_of:
                    emit_T(wT, atok_of[wT])
                    del atok_of[wT]
                if 0 <= wE:
                    emit_E(wE)

    nc.compile()
    return nc


def _get_nc():
    if "nc" not in _CACHE:
        _CACHE["nc"] = _build()
    return _CACHE["nc"]


def make_in_maps(x, w_qkv, w_out):
    x = np.asarray(x, dtype=np.float32)
    w_qkv_b = np.asarray(w_qkv, dtype=np.float32).astype(ml_dtypes.bfloat16)
    w_out_b = np.asarray(w_out, dtype=np.float32).astype(ml_dtypes.bfloat16)

    # mask2[j, c, i] = 1 where j <= i, replicated for both heads of a group
    maskT = np.triu(np.ones((P, P), dtype=np.float32))
    mask2 = np.broadcast_to(maskT[:, None, :], (P, 2, P)).astype(ml_dtypes.bfloat16)
    mask2 = np.ascontiguousarray(mask2)
    ident = np.eye(P, dtype=np.float32).astype(ml_dtypes.bfloat16)

    x_pad = np.concatenate([np.zeros((WSZ, DIM), np.float32), x], axis=0)
    in_maps = []
    for c in range(NCORES):
        x_sh = x_pad[c * R:c * R + T]                       # (2176, 1024)
        xT = np.ascontiguousarray(x_sh.T).astype(ml_dtypes.bfloat16)
        in_maps.append({
            "xT": xT,
            "wqkv": w_qkv_b,
            "wout": w_out_b,
            "mask2": mask2,
            "ident": ident,
        })
    return in_maps


def kernel(x, w_qkv, w_out, b_out):
    b_out = np.asarray(b_out, dtype=np.float32)
    in_maps = make_in_maps(x, w_qkv, w_out)
    nc = _get_nc()
    res = run_bass_kernel_spmd(nc, in_maps, core_ids=list(range(NCORES)))
    out = np.concatenate(
        [res.results[c]["out"].astype(np.float32) for c in range(NCORES)], axis=0
    )
    return out + b_out[None, :]
